# revision 1
# baseline (speedup 1.0000x reference)
"""Trainium2 Bass kernel for a GCN message-passing layer.

Reference computation (per node i):
    out[i] = sum_j edges[i,j] * (w1 @ concat(x[j], dist[i,j])) + w2 @ x[i]
which factors into:
    xmsg = x @ w1x.T                       (w1x = w1[:, :128])
    agg  = edges @ xmsg                    (big GEMM, contraction over j)
    dw   = einsum('ij,ijc->ic', edges, dist)
    out  = agg + dw @ w1d.T + x @ w2.T     (w1d = w1[:, 128:130])

Sharding: rows i (targets) split across 8 NeuronCores; x/w1/w2 replicated.
Each core streams its [1024, 8192] edges slice and [1024, 8192, 2] dist
slice from HBM exactly once (~100 MB/core -> memory-bound, ~295 us floor
at the ~358 GB/s per-core HBM limit).

Device layout: everything is computed transposed, out^T[f, i], so the
moving matmul operand is E^T tiles (built on-chip with PE transposes,
128x128 each) and xmsg[j, f] chunks act as stationary weights. The dist
einsum runs on the vector engine as scalar_tensor_tensor with accum_out
(fused multiply + free-dim reduce, one instruction per (tile, channel));
its rank-2 result folds into the same PSUM accumulation via small K=2
matmuls spread across the loop, so there is no serialized epilogue.

DMA plan: the sync HWDGE ring carries nothing but the 0.5 MB / 1 MB
edge/dist stream (~360 GB/s sustained, triple-buffered). Prologue loads ride the scalar
ring; output stores ride the GPSIMD SWDGE ring (a waiting store must
never block load triggers). The last granule is split into half-width
pieces so the final reductions pipeline with data arrival. The xmsg
prologue batches 4 matmuls per PSUM bank with one copy so it drains
quickly under the DMA runway. Host transposes the final [128, 1024]
per-core result.
"""

import os

import numpy as np

import concourse.bacc as bacc
import concourse.mybir as mybir
from concourse.tile import TileContext
from concourse.masks import make_identity

F32 = mybir.dt.float32
P = 128

# problem dims (hardcoded per contract)
N_FULL = 8192
F_IN = 128
F_OUT = 128
N_CORES = 8

LAST_RESULT = None  # BassKernelResults of the most recent kernel() call


def _granules(n_jb, jb, is_last_isup):
    """Granule schedule for one i-supertile: (jblk, joff, width) tuples.

    The last i-supertile's final granule is split in half so the kernel
    tail (compute on the last-arriving data) is half as long.
    """
    g = [(jblk, 0, jb) for jblk in range(n_jb)]
    if is_last_isup and jb >= 8 * P:
        g[-1:] = [
            (n_jb - 1, 0, jb // 2),
            (n_jb - 1, jb // 2, jb // 4),
            (n_jb - 1, 3 * jb // 4, jb // 4),
        ]
    elif is_last_isup and jb >= 4 * P:
        g[-1:] = [(n_jb - 1, 0, jb // 2), (n_jb - 1, jb // 2, jb // 2)]
    return g


def build(n=N_FULL, rows=N_FULL // N_CORES, jb=1024, ni=256):
    """Build the per-core SPMD Bass program.

    n:    number of source nodes j (columns of edges)
    rows: number of target rows i this core handles
    jb:   j-block width streamed per DMA granule
    ni:   i-supertile width (output columns accumulated per PSUM group)
    """
    f = F_IN
    assert n % jb == 0 and rows % ni == 0 and ni % P == 0 and jb % (4 * P) == 0
    assert ni <= 512
    n_jb = n // jb
    n_isup = rows // ni
    n_ib = ni // P
    tch = 512 // ni  # j-chunks per transpose-staging bank

    nc = bacc.Bacc()
    xT_d = nc.declare_dram_parameter("xT", [f, n], F32, isOutput=False)
    xTs_d = nc.declare_dram_parameter("xT_self", [f, rows], F32, isOutput=False)
    e_d = nc.declare_dram_parameter("edges", [rows, n], F32, isOutput=False)
    d_d = nc.declare_dram_parameter("dist", [rows, n, 2], F32, isOutput=False)
    w1xT_d = nc.declare_dram_parameter("w1xT", [f, F_OUT], F32, isOutput=False)
    w2T_d = nc.declare_dram_parameter("w2T", [f, F_OUT], F32, isOutput=False)
    w1dT_d = nc.declare_dram_parameter("w1dT", [2, F_OUT], F32, isOutput=False)
    o_d = nc.declare_dram_parameter("outT", [F_OUT, rows], F32, isOutput=True)

    with TileContext(nc) as tc:
        with (
            tc.tile_pool(name="const", bufs=1) as cpool,
            tc.tile_pool(name="stream", bufs=2) as pool,
            tc.tile_pool(name="psum", bufs=2, space="PSUM") as pp,
        ):
            def load_granule(isup, gi, jblk, joff, w):
                e_t, d_t = [], []
                for ib in range(n_ib):
                    i_blk = isup * n_ib + ib
                    et = pool.tile(
                        [P, w], F32, tag=f"E{ib}", bufs=3,
                        name=f"et{ib}_{isup}_{gi}",
                    )
                    nc.sync.dma_start(
                        et,
                        e_d[
                            i_blk * P : (i_blk + 1) * P,
                            jblk * jb + joff : jblk * jb + joff + w,
                        ],
                    )
                    dt = pool.tile(
                        [P, w, 2], F32, tag=f"D{ib}", bufs=3,
                        name=f"dt{ib}_{isup}_{gi}",
                    )
                    nc.sync.dma_start(
                        dt,
                        d_d[
                            i_blk * P : (i_blk + 1) * P,
                            jblk * jb + joff : jblk * jb + joff + w,
                            :,
                        ],
                    )
                    e_t.append(et)
                    d_t.append(dt)
                return e_t, d_t

            # issue the first granule's loads before anything else so the
            # sync ring starts streaming at t=0
            pre = {(0, 0): load_granule(0, 0, 0, 0, jb)}

            # ---------------- prologue ----------------
            # weight loads go out BEFORE make_identity: affine_select's
            # first use can pay a ~6 us Q7 IRAM load that would otherwise
            # delay these same-engine DMA triggers (w2T gates the first
            # PE instruction of the main loop)
            xTs_sb = cpool.tile([f, rows], F32)
            nc.gpsimd.dma_start(xTs_sb, xTs_d[:, :])
            w1xT = cpool.tile([f, F_OUT], F32)
            nc.gpsimd.dma_start(w1xT, w1xT_d[:, :])
            w2T = cpool.tile([f, F_OUT], F32)
            nc.gpsimd.dma_start(w2T, w2T_d[:, :])
            w1dT = cpool.tile([2, F_OUT], F32)
            nc.gpsimd.dma_start(w1dT, w1dT_d[:, :])

            ident = cpool.tile([P, P], F32)
            make_identity(nc, ident)

            # xT split so the xmsg matmuls can start as soon as the first
            # piece lands (scalar HWDGE ring)
            xTp = []
            for b in range(n_jb):
                t = cpool.tile([f, jb], F32, name=f"xTp{b}")
                nc.scalar.dma_start(t, xT_d[:, b * jb : (b + 1) * jb])
                xTp.append(t)

            # xmsg[j, f] chunks, 4 matmuls per PSUM bank + one batched copy
            xmsg = cpool.tile([P, n // P, f], F32)
            for q in range(n // P // 4):
                xm = pp.tile([P, 512], F32, tag="tstage", bufs=4)
                for r in range(4):
                    ch = 4 * q + r
                    b, off = divmod(ch * P, jb)
                    nc.tensor.matmul(
                        xm[:, r * P : (r + 1) * P],
                        xTp[b][:, off : off + P],
                        w1xT,
                        start=True,
                        stop=True,
                    )
                nc.any.tensor_copy(xmsg[:, 4 * q : 4 * q + 4], xm)

            dummy = cpool.tile([P, 1], F32)  # sink for STT streams

            # ---------------- main loop ----------------
            for isup in range(n_isup):
                agg = pp.tile([P, ni], F32, tag="agg")

                # self-connection term: out^T += w2 @ x_self^T
                nc.tensor.matmul(
                    agg,
                    w2T,
                    xTs_sb[:, isup * ni : (isup + 1) * ni],
                    start=True,
                    stop=False,
                )

                grans = _granules(n_jb, jb, isup == n_isup - 1)
                for gi, (jblk, joff, w) in enumerate(grans):
                    if (isup, gi) in pre:
                        e_t, d_t = pre.pop((isup, gi))
                    else:
                        e_t, d_t = load_granule(isup, gi, jblk, joff, w)

                    # fused multiply+reduce: dwp[ib][:, c] = sum_j E*D_c
                    # (scalar_tensor_tensor = standard TensorScalarPtr op;
                    # accum_out yields the free-dim sum for free)
                    dwp = []
                    for ib in range(n_ib):
                        dwt = pool.tile(
                            [P, 2], F32, tag=f"dwp{ib}", bufs=3,
                            name=f"dwp{ib}_{isup}_{gi}",
                        )
                        for c in range(2):
                            nc.vector.scalar_tensor_tensor(
                                dummy.broadcast_to((P, w)),
                                e_t[ib],
                                1.0,
                                d_t[ib][:, :, c],
                                op0=mybir.AluOpType.mult,
                                op1=mybir.AluOpType.mult,
                                accum_out=dwt[:, c : c + 1],
                            )
                        dwp.append(dwt)

                    # E^T tiles via PE transposes (tch j-chunks per staging
                    # bank, one batched copy), then the accumulating matmuls
                    for g2 in range(w // (tch * P)):
                        tpt = pp.tile([P, tch * ni], F32, tag="tstage", bufs=4)
                        for h in range(tch):
                            for ib in range(n_ib):
                                col = (h * n_ib + ib) * P
                                src = (g2 * tch + h) * P
                                nc.tensor.transpose(
                                    tpt[:, col : col + P],
                                    e_t[ib][:, src : src + P],
                                    ident,
                                )
                        ett = pool.tile([P, tch * ni], F32, tag="Et", bufs=3)
                        nc.any.tensor_copy(ett, tpt)
                        for h in range(tch):
                            jglob = (jblk * jb + joff) // P + g2 * tch + h
                            nc.tensor.matmul(
                                agg,
                                xmsg[:, jglob],
                                ett[:, h * ni : (h + 1) * ni],
                                start=False,
                                stop=False,
                            )

                    # distance-feature term for this granule:
                    # out^T += w1d @ dwp^T (K=2 matmul, spread across the
                    # loop so nothing big serializes after the last granule)
                    tpq = pp.tile([2, ni], F32, tag="tstage", bufs=4)
                    for ib in range(n_ib):
                        nc.tensor.transpose(
                            tpq[:, ib * P : (ib + 1) * P], dwp[ib], ident
                        )
                    dwT = pool.tile([2, ni], F32, tag="dwT", bufs=3)
                    nc.any.tensor_copy(dwT, tpq)
                    nc.tensor.matmul(
                        agg, w1dT, dwT, start=False, stop=(gi == len(grans) - 1)
                    )

                out_sb = pool.tile([P, ni], F32, tag="osb")
                nc.any.tensor_copy(out_sb, agg)
                # final store rides the now-idle sync ring; earlier ones go
                # out on SWDGE so a waiting store never blocks load triggers
                store_eng = nc.sync if isup == n_isup - 1 else nc.gpsimd
                store_eng.dma_start(o_d[:, isup * ni : (isup + 1) * ni], out_sb)

    nc.compile()
    return nc


def _run(inputs, n, rows_per_core, n_cores, jb, ni, trace=False):
    from concourse.bass_utils import run_bass_kernel_spmd

    x = np.ascontiguousarray(np.asarray(inputs["x"], dtype=np.float32))
    edges = np.asarray(inputs["edges"], dtype=np.float32)
    dist = np.asarray(inputs["distance_matrix"], dtype=np.float32)
    w1 = np.ascontiguousarray(np.asarray(inputs["w1"], dtype=np.float32))
    w2 = np.ascontiguousarray(np.asarray(inputs["w2"], dtype=np.float32))

    xT = np.ascontiguousarray(x.T)
    w1xT = np.ascontiguousarray(w1[:, : x.shape[1]].T)
    w2T = np.ascontiguousarray(w2.T)
    w1dT = np.ascontiguousarray(w1[:, x.shape[1] :].T)

    in_maps = []
    for c in range(n_cores):
        i0 = c * rows_per_core
        i1 = i0 + rows_per_core
        in_maps.append(
            {
                "xT": xT,
                "xT_self": np.ascontiguousarray(xT[:, i0:i1]),
                "edges": np.ascontiguousarray(edges[i0:i1]),
                "dist": np.ascontiguousarray(dist[i0:i1]),
                "w1xT": w1xT,
                "w2T": w2T,
                "w1dT": w1dT,
            }
        )

    nc = build(n=n, rows=rows_per_core, jb=jb, ni=ni)
    res = run_bass_kernel_spmd(nc, in_maps, core_ids=list(range(n_cores)), trace=trace)

    global LAST_RESULT
    LAST_RESULT = res

    out = np.concatenate([r["outT"].T for r in res.results], axis=0)
    return out


def kernel(**inputs) -> np.ndarray:
    trace = os.environ.get("KERNEL_TRACE", "0") == "1"
    return _run(
        inputs,
        n=N_FULL,
        rows_per_core=N_FULL // N_CORES,
        n_cores=N_CORES,
        jb=1024,
        ni=256,
        trace=trace,
    )



# revision 2
# speedup vs baseline: 1.3859x; 1.3859x over previous
"""Trainium2 Bass kernel for a GCN message-passing layer.

Reference computation (per node i):
    out[i] = sum_j edges[i,j] * (w1 @ concat(x[j], dist[i,j])) + w2 @ x[i]
which factors into:
    xmsg = x @ w1x.T                       (w1x = w1[:, :128])
    agg  = edges @ xmsg                    (big GEMM, contraction over j)
    dw   = einsum('ij,ijc->ic', edges, dist)
    out  = agg + dw @ w1d.T + x @ w2.T     (w1d = w1[:, 128:130])

Sharding: rows i (targets) split across 8 NeuronCores; x/w1/w2 replicated.

Device strategy (v2, ~2x the v1 fp32 baseline):
  - All big streams are converted to bf16 on the host (tolerance is 2e-2
    rel-L2; bf16 streams land ~3e-3).  Per-core HBM traffic drops from
    ~100 MB (fp32) to ~50 MB -> ~140 us at the ~358 GB/s per-core HBM
    ceiling.
  - edges and dist are PRE-TRANSPOSED on the host to [j, i] layout
    (contraction dim j on partitions).  The PE then consumes E^T tiles
    directly as the moving operand -- no on-chip transposes, no
    PSUM->SBUF staging copies at all.
  - out^T[f, i] = sum_j xmsg[j, f] * E^T[j, i]  accumulates in PSUM with
    xmsg chunks as stationary weights (bf16 matmul = 1 cycle/col vs 4
    for fp32).
  - dist term: DVE computes prod_c = E^T (*) D^T_c elementwise (bf16 ->
    2x perf mode), and the PE reduces over j by a matmul with a
    broadcast-w1d stationary (w1db_c[j, f] = w1d[f, c] for all j), which
    folds the whole einsum+w1d GEMM into the same PSUM accumulation.
  - self term: out^T += w2^T.T @ x_self^T, same PSUM group.

DMA plan: the sync HWDGE ring carries only the E^T/D^T stream (one
256 KB + one 512 KB contiguous chunk per 128-row j-tile, triple
buffered).  Prologue (x^T pieces for the xmsg GEMM, weights) rides the
scalar ring.  The single 512 KB out^T store rides the sync ring after
the stream has drained.
"""

import os

import numpy as np
import ml_dtypes

import concourse.bacc as bacc
import concourse.mybir as mybir
from concourse.tile import TileContext

F32 = mybir.dt.float32
BF16 = mybir.dt.bfloat16
P = 128

# problem dims (hardcoded per contract)
N_FULL = 8192
F_IN = 128
F_OUT = 128
N_CORES = 8

LAST_RESULT = None  # BassKernelResults of the most recent kernel() call


def build(n=N_FULL, rows=N_FULL // N_CORES):
    """Build the per-core SPMD Bass program.

    n:    number of source nodes j (contraction dim, partition-tiled)
    rows: number of target rows i this core handles (free dim of out^T)
    """
    f = F_IN
    assert n % P == 0 and rows % 512 == 0
    n_jt = n // P          # 128-row j-tiles streamed from HBM
    n_h = rows // 512      # 512-col PSUM halves of out^T
    XCH = 1024             # x^T prologue piece width (j columns)

    nc = bacc.Bacc()
    eT_d = nc.declare_dram_parameter("eT", [n, rows], BF16, isOutput=False)
    dT_d = nc.declare_dram_parameter("dT", [n, 2, rows], BF16, isOutput=False)
    xT_d = nc.declare_dram_parameter("xT", [f, n], BF16, isOutput=False)
    xTs_d = nc.declare_dram_parameter("xT_self", [f, rows], BF16, isOutput=False)
    w1xT_d = nc.declare_dram_parameter("w1xT", [f, F_OUT], BF16, isOutput=False)
    w2T_d = nc.declare_dram_parameter("w2T", [f, F_OUT], BF16, isOutput=False)
    w1db_d = nc.declare_dram_parameter("w1db", [P, 2, F_OUT], BF16, isOutput=False)
    o_d = nc.declare_dram_parameter("outT", [F_OUT, rows], F32, isOutput=True)

    with TileContext(nc) as tc:
        with (
            tc.tile_pool(name="const", bufs=1) as cpool,
            tc.tile_pool(name="stream", bufs=2) as pool,
            tc.tile_pool(name="psum", bufs=2, space="PSUM") as pp,
        ):
            def load_jtile(jt):
                et = pool.tile([P, rows], BF16, tag="E", bufs=3, name=f"et{jt}")
                nc.sync.dma_start(et, eT_d[jt * P : (jt + 1) * P, :])
                dt = pool.tile([P, 2, rows], BF16, tag="D", bufs=3, name=f"dt{jt}")
                nc.sync.dma_start(dt, dT_d[jt * P : (jt + 1) * P, :, :])
                return et, dt

            # issue the first tiles' loads before anything else so the
            # sync ring starts streaming at t=0
            pre = {jt: load_jtile(jt) for jt in range(2)}

            # ---------------- prologue (scalar ring) ----------------
            xTs_sb = cpool.tile([f, rows], BF16)
            nc.scalar.dma_start(xTs_sb, xTs_d[:, :])
            w2T = cpool.tile([f, F_OUT], BF16)
            nc.scalar.dma_start(w2T, w2T_d[:, :])
            w1xT = cpool.tile([f, F_OUT], BF16)
            nc.scalar.dma_start(w1xT, w1xT_d[:, :])
            w1db = cpool.tile([P, 2, F_OUT], BF16)
            nc.scalar.dma_start(w1db, w1db_d[:, :, :])

            # x^T pieces so the xmsg GEMM can start as soon as each lands
            xTp = []
            for b in range(n // XCH):
                t = cpool.tile([f, XCH], BF16, name=f"xTp{b}")
                nc.scalar.dma_start(t, xT_d[:, b * XCH : (b + 1) * XCH])
                xTp.append(t)

            # xmsg[j, f] stationary chunks: 4 matmuls per PSUM bank then
            # one batched bf16 copy (DVE, 2x)
            xmsg = cpool.tile([P, n // P, f], BF16)
            for q in range(n // P // 4):
                xm = pp.tile([P, 4, f], F32, tag="xstage", bufs=4)
                for r in range(4):
                    ch = 4 * q + r
                    b, off = divmod(ch * P, XCH)
                    nc.tensor.matmul(
                        xm[:, r],
                        xTp[b][:, off : off + P],
                        w1xT,
                        start=True,
                        stop=True,
                    )
                nc.vector.tensor_copy(xmsg[:, 4 * q : 4 * q + 4], xm)

            # ---------------- main loop ----------------
            # out^T accumulates in two persistent PSUM halves (512 cols
            # of fp32 each = one PSUM bank)
            aggs = [
                pp.tile([P, 512], F32, tag=f"agg{h}", bufs=1, name=f"agg{h}")
                for h in range(n_h)
            ]
            # self-connection term opens each accumulation group
            for h in range(n_h):
                nc.tensor.matmul(
                    aggs[h],
                    w2T,
                    xTs_sb[:, h * 512 : (h + 1) * 512],
                    start=True,
                    stop=False,
                )

            for jt in range(n_jt):
                if jt in pre:
                    et, dt = pre.pop(jt)
                else:
                    et, dt = load_jtile(jt)

                # dist products on DVE (bf16 2x mode, unit stride)
                prod = pool.tile([P, 2, rows], BF16, tag="PR", bufs=3,
                                 name=f"prod{jt}")
                nc.vector.tensor_tensor(
                    prod[:, 0], et, dt[:, 0], op=mybir.AluOpType.mult
                )
                nc.vector.tensor_tensor(
                    prod[:, 1], et, dt[:, 1], op=mybir.AluOpType.mult
                )

                last = jt == n_jt - 1
                # one stationary load each, then both 512-col halves
                for h in range(n_h):
                    nc.tensor.matmul(
                        aggs[h],
                        xmsg[:, jt],
                        et[:, h * 512 : (h + 1) * 512],
                        start=False,
                        stop=False,
                    )
                for c in range(2):
                    for h in range(n_h):
                        nc.tensor.matmul(
                            aggs[h],
                            w1db[:, c],
                            prod[:, c, h * 512 : (h + 1) * 512],
                            start=False,
                            stop=last and c == 1,
                        )

            # ---------------- epilogue ----------------
            out_sb = pool.tile([P, rows], F32, tag="osb", bufs=1)
            for h in range(n_h):
                nc.vector.tensor_copy(out_sb[:, h * 512 : (h + 1) * 512], aggs[h])
            nc.sync.dma_start(o_d[:, :], out_sb)

    nc.compile()
    return nc


def _prep_inputs(inputs, n, rows_per_core, n_cores):
    """Host-side shard + layout + dtype prep (numpy only)."""
    bf16 = ml_dtypes.bfloat16
    x = np.asarray(inputs["x"], dtype=np.float32)
    edges = np.asarray(inputs["edges"], dtype=np.float32)
    dist = np.asarray(inputs["distance_matrix"], dtype=np.float32)
    w1 = np.asarray(inputs["w1"], dtype=np.float32)
    w2 = np.asarray(inputs["w2"], dtype=np.float32)
    f = x.shape[1]

    xT = np.ascontiguousarray(x.T).astype(bf16)            # [f, n]
    w1xT = np.ascontiguousarray(w1[:, :f].T).astype(bf16)  # [k, F_OUT]
    w2T = np.ascontiguousarray(w2.T).astype(bf16)          # [k, F_OUT]
    # broadcast-w1d stationaries: w1db[j, c, fout] = w1[fout, f+c]
    w1db = np.ascontiguousarray(
        np.broadcast_to(w1[:, f : f + 2].T[None, :, :], (P, 2, w1.shape[0]))
    ).astype(bf16)

    in_maps = []
    for c in range(n_cores):
        i0 = c * rows_per_core
        i1 = i0 + rows_per_core
        # E^T slice: [n, rows]  (edges[i, j] -> eT[j, i])
        eT = np.ascontiguousarray(edges[i0:i1].T).astype(bf16)
        # D^T channel-packed: dT[j, c, i] = dist[i, j, c]
        dT = np.ascontiguousarray(dist[i0:i1].transpose(1, 2, 0)).astype(bf16)
        in_maps.append(
            {
                "eT": eT,
                "dT": dT,
                "xT": xT,
                "xT_self": np.ascontiguousarray(xT[:, i0:i1]),
                "w1xT": w1xT,
                "w2T": w2T,
                "w1db": w1db,
            }
        )
    return in_maps


def _run(inputs, n, rows_per_core, n_cores, trace=False):
    from concourse.bass_utils import run_bass_kernel_spmd

    in_maps = _prep_inputs(inputs, n, rows_per_core, n_cores)
    nc = build(n=n, rows=rows_per_core)
    res = run_bass_kernel_spmd(nc, in_maps, core_ids=list(range(n_cores)), trace=trace)

    global LAST_RESULT
    LAST_RESULT = res

    out = np.concatenate([r["outT"].T for r in res.results], axis=0)
    return np.ascontiguousarray(out, dtype=np.float32)


def kernel(**inputs) -> np.ndarray:
    trace = os.environ.get("KERNEL_TRACE", "0") == "1"
    return _run(
        inputs,
        n=N_FULL,
        rows_per_core=N_FULL // N_CORES,
        n_cores=N_CORES,
        trace=trace,
    )


# revision 6
# speedup vs baseline: 1.9804x; 1.4289x over previous
"""Trainium2 Bass kernel for a GCN message-passing layer.

Reference computation (per node i):
    out[i] = sum_j edges[i,j] * (w1 @ concat(x[j], dist[i,j])) + w2 @ x[i]
which factors into:
    xmsg = x @ w1x.T                       (w1x = w1[:, :128])
    agg  = edges @ xmsg                    (big GEMM, contraction over j)
    dw   = einsum('ij,ijc->ic', edges, dist)
    out  = agg + dw @ w1d.T + x @ w2.T     (w1d = w1[:, 128:130])

Sharding: rows i (targets) split across 8 NeuronCores; x/w1/w2 replicated.

Device strategy (tolerance is 2e-2 rel-L2; this kernel lands ~5e-3):
  - edges stream as bf16, dist channels as fp8-e4m3 (dist only feeds the
    small dw term, fp8 keeps its error contribution ~3e-3).  Per-core
    HBM traffic drops from ~100 MB fp32 to ~34 MB -> ~95 us at the
    ~358 GB/s per-core HBM ceiling.
  - edges and dist are PRE-TRANSPOSED on the host to [j, i] layout
    (contraction dim j on partitions).  The PE consumes E^T tiles
    directly as the moving operand -- no on-chip transposes, no
    PSUM->SBUF staging copies.
  - out^T[f, i] = sum_j xmsg[j, f] * E^T[j, i] accumulates in a single
    [128, 1024] fp32 PSUM tile; bf16 matmul moving operand is 1024 cols
    at 1 cycle/col (fp32 would be 4x slower).
  - dist term: DVE computes prod = E^T (*) D^T for both channels in ONE
    tensor_tensor (channel-broadcast AP on E^T, bf16 2x perf mode), and
    the PE folds the j-reduction AND the w1d GEMM into the same PSUM
    accumulation via broadcast-w1d stationaries
    (w1db_c[j, f] = w1d[f, c] for all j).
  - fp8->bf16 upconversion of dist runs on the otherwise-idle ScalarE
    for most tiles; every DVE_EVERY-th tile instead multiplies the fp8
    operand directly on DVE (mixed-dtype TT at 1x) to balance the two
    engines.  Dist-matmul stationaries are batched across tile pairs to
    halve LDWEIGHTS traffic.
"""

import os

import numpy as np
import ml_dtypes

import concourse.bacc as bacc
import concourse.mybir as mybir
from concourse.tile import TileContext

F32 = mybir.dt.float32
BF16 = mybir.dt.bfloat16
FP8 = mybir.dt.float8e4
P = 128

# problem dims (hardcoded per contract)
N_FULL = 8192
F_IN = 128
F_OUT = 128
N_CORES = 8

# tunables
DVE_EVERY = 5       # every k-th j-tile multiplies fp8 directly on DVE
DIST_BATCH = 2      # j-tiles sharing one w1db stationary load
STREAM_BUFS = 6     # deep buffering so the DMA stream free-runs

LAST_RESULT = None  # BassKernelResults of the most recent kernel() call


def build(n=N_FULL, rows=N_FULL // N_CORES):
    """Build the per-core SPMD Bass program.

    n:    number of source nodes j (contraction dim, partition-tiled)
    rows: number of target rows i this core handles (free dim of out^T)
    """
    f = F_IN
    assert n % P == 0 and rows == 1024
    n_jt = n // P          # 128-row j-tiles streamed from HBM
    XCH = 1024             # x^T prologue piece width (j columns)

    nc = bacc.Bacc()
    eT_d = nc.declare_dram_parameter("eT", [n, rows], BF16, isOutput=False)
    dT_d = nc.declare_dram_parameter("dT", [n, 2, rows], FP8, isOutput=False)
    xT_d = nc.declare_dram_parameter("xT", [f, n], BF16, isOutput=False)
    xTs_d = nc.declare_dram_parameter("xT_self", [f, rows], BF16, isOutput=False)
    w1xT_d = nc.declare_dram_parameter("w1xT", [f, F_OUT], BF16, isOutput=False)
    w2T_d = nc.declare_dram_parameter("w2T", [f, F_OUT], BF16, isOutput=False)
    w1db_d = nc.declare_dram_parameter("w1db", [P, 2, F_OUT], BF16, isOutput=False)
    o_d = nc.declare_dram_parameter("outT", [F_OUT, rows], F32, isOutput=True)

    with TileContext(nc) as tc:
        with (
            tc.tile_pool(name="const", bufs=1) as cpool,
            tc.tile_pool(name="stream", bufs=2) as pool,
            tc.tile_pool(name="psum", bufs=2, space="PSUM") as pp,
        ):
            def load_jtile(jt):
                et = pool.tile([P, rows], BF16, tag="E", bufs=STREAM_BUFS,
                               name=f"et{jt}")
                nc.sync.dma_start(et, eT_d[jt * P : (jt + 1) * P, :])
                d8 = pool.tile([P, 2, rows], FP8, tag="D", bufs=STREAM_BUFS,
                               name=f"d8_{jt}")
                nc.sync.dma_start(d8, dT_d[jt * P : (jt + 1) * P, :, :])
                return et, d8

            # issue the first tiles' loads before anything else so the
            # sync ring starts streaming at t=0
            pre = {jt: load_jtile(jt) for jt in range(4)}

            # ---------------- prologue (scalar ring) ----------------
            xTs_sb = cpool.tile([f, rows], BF16)
            nc.scalar.dma_start(xTs_sb, xTs_d[:, :])
            w2T = cpool.tile([f, F_OUT], BF16)
            nc.scalar.dma_start(w2T, w2T_d[:, :])
            w1xT = cpool.tile([f, F_OUT], BF16)
            nc.scalar.dma_start(w1xT, w1xT_d[:, :])
            w1db = cpool.tile([P, 2, F_OUT], BF16)
            nc.scalar.dma_start(w1db, w1db_d[:, :, :])

            # x^T pieces so the xmsg GEMM can start as soon as each lands
            xTp = []
            for b in range(n // XCH):
                t = cpool.tile([f, XCH], BF16, name=f"xTp{b}")
                nc.scalar.dma_start(t, xT_d[:, b * XCH : (b + 1) * XCH])
                xTp.append(t)

            # xmsg[j, f] stationary chunks: 4 matmuls per PSUM bank then
            # one batched bf16 copy, alternating copy engine
            xmsg = cpool.tile([P, n // P, f], BF16)
            for q in range(n // P // 4):
                xm = pp.tile([P, 4, f], F32, tag="xstage", bufs=4)
                for r in range(4):
                    ch = 4 * q + r
                    b, off = divmod(ch * P, XCH)
                    nc.tensor.matmul(
                        xm[:, r],
                        xTp[b][:, off : off + P],
                        w1xT,
                        start=True,
                        stop=True,
                    )
                eng = nc.vector if q % 2 == 0 else nc.scalar
                if eng is nc.vector:
                    eng.tensor_copy(xmsg[:, 4 * q : 4 * q + 4], xm)
                else:
                    eng.copy(xmsg[:, 4 * q : 4 * q + 4], xm)

            # ---------------- main loop ----------------
            agg = pp.tile([P, rows], F32, tag="agg", bufs=1, name="agg")
            # self-connection term opens each half's accumulation group
            # (single matmul may not span a PSUM bank -> 512-col halves)
            for h in range(2):
                sl = slice(h * 512, (h + 1) * 512)
                nc.tensor.matmul(
                    agg[:, sl], w2T, xTs_sb[:, sl], start=True, stop=False
                )

            def do_tile(jt, et, d8):
                """DVE/Scalar work for one tile; returns prod tile."""
                prod = pool.tile([P, 2, rows], BF16, tag="PR",
                                 bufs=2 * DIST_BATCH, name=f"prod{jt}")
                ebc = et[:, None, :].broadcast_to((P, 2, rows))
                if jt % DVE_EVERY == DVE_EVERY - 1:
                    # direct mixed-dtype TT (fp8 operand, 1x mode)
                    nc.vector.tensor_tensor(
                        prod, ebc, d8, op=mybir.AluOpType.mult
                    )
                else:
                    db = pool.tile([P, 2, rows], BF16, tag="DB", bufs=3,
                                   name=f"db{jt}")
                    nc.scalar.copy(db, d8)
                    nc.vector.tensor_tensor(
                        prod, ebc, db, op=mybir.AluOpType.mult
                    )
                return prod

            for g0 in range(0, n_jt, DIST_BATCH):
                group = range(g0, min(g0 + DIST_BATCH, n_jt))
                prods = []
                for jt in group:
                    et, d8 = pre.pop(jt) if jt in pre else load_jtile(jt)
                    prods.append(do_tile(jt, et, d8))
                    # agg matmuls (stationary = xmsg chunk jt, reused for
                    # both 512-col halves)
                    for h in range(2):
                        sl = slice(h * 512, (h + 1) * 512)
                        nc.tensor.matmul(
                            agg[:, sl],
                            xmsg[:, jt],
                            et[:, sl],
                            start=False,
                            stop=False,
                        )
                last_group = g0 + DIST_BATCH >= n_jt
                for c in range(2):
                    for k, jt in enumerate(group):
                        for h in range(2):
                            sl = slice(h * 512, (h + 1) * 512)
                            nc.tensor.matmul(
                                agg[:, sl],
                                w1db[:, c],
                                prods[k][:, c, sl],
                                start=False,
                                stop=last_group and c == 1 and jt == n_jt - 1,
                            )

            # ---------------- epilogue ----------------
            out_sb = pool.tile([P, rows], F32, tag="osb", bufs=1)
            for hh in range(2):
                sl = slice(hh * 512, (hh + 1) * 512)
                nc.scalar.copy(out_sb[:, sl], agg[:, sl])
                nc.sync.dma_start(o_d[:, sl], out_sb[:, sl])

    nc.compile()
    return nc


def _prep_inputs(inputs, n, rows_per_core, n_cores):
    """Host-side shard + layout + dtype prep (numpy only)."""
    bf16 = ml_dtypes.bfloat16
    # e4m3fn: bit-identical to TRN FP8_EXP4 for |x| <= 240 (all our data),
    # and the fn variant is the one the PJRT plugin accepts as input type
    fp8 = ml_dtypes.float8_e4m3fn
    x = np.asarray(inputs["x"], dtype=np.float32)
    edges = np.asarray(inputs["edges"], dtype=np.float32)
    dist = np.asarray(inputs["distance_matrix"], dtype=np.float32)
    w1 = np.asarray(inputs["w1"], dtype=np.float32)
    w2 = np.asarray(inputs["w2"], dtype=np.float32)
    f = x.shape[1]

    xT = np.ascontiguousarray(x.T).astype(bf16)            # [f, n]
    w1xT = np.ascontiguousarray(w1[:, :f].T).astype(bf16)  # [k, F_OUT]
    w2T = np.ascontiguousarray(w2.T).astype(bf16)          # [k, F_OUT]
    # broadcast-w1d stationaries: w1db[j, c, fout] = w1[fout, f+c]
    w1db = np.ascontiguousarray(
        np.broadcast_to(w1[:, f : f + 2].T[None, :, :], (P, 2, w1.shape[0]))
    ).astype(bf16)

    in_maps = []
    for c in range(n_cores):
        i0 = c * rows_per_core
        i1 = i0 + rows_per_core
        # E^T slice: [n, rows]  (edges[i, j] -> eT[j, i])
        eT = np.ascontiguousarray(edges[i0:i1].T).astype(bf16)
        # D^T channel-packed: dT[j, c, i] = dist[i, j, c]
        dT = np.ascontiguousarray(dist[i0:i1].transpose(1, 2, 0)).astype(fp8)
        in_maps.append(
            {
                "eT": eT,
                "dT": dT,
                "xT": xT,
                "xT_self": np.ascontiguousarray(xT[:, i0:i1]),
                "w1xT": w1xT,
                "w2T": w2T,
                "w1db": w1db,
            }
        )
    return in_maps


def _run(inputs, n, rows_per_core, n_cores, trace=False):
    from concourse.bass_utils import run_bass_kernel_spmd

    in_maps = _prep_inputs(inputs, n, rows_per_core, n_cores)
    nc = build(n=n, rows=rows_per_core)
    res = run_bass_kernel_spmd(nc, in_maps, core_ids=list(range(n_cores)), trace=trace)

    global LAST_RESULT
    LAST_RESULT = res

    out = np.concatenate([r["outT"].T for r in res.results], axis=0)
    return np.ascontiguousarray(out, dtype=np.float32)


def kernel(**inputs) -> np.ndarray:
    trace = os.environ.get("KERNEL_TRACE", "0") == "1"
    return _run(
        inputs,
        n=N_FULL,
        rows_per_core=N_FULL // N_CORES,
        n_cores=N_CORES,
        trace=trace,
    )


# revision 7
# speedup vs baseline: 1.9927x; 1.0062x over previous
"""Trainium2 Bass kernel for a GCN message-passing layer.

Reference computation (per node i):
    out[i] = sum_j edges[i,j] * (w1 @ concat(x[j], dist[i,j])) + w2 @ x[i]
which factors into:
    xmsg = x @ w1x.T                       (w1x = w1[:, :128])
    agg  = edges @ xmsg                    (big GEMM, contraction over j)
    dw   = einsum('ij,ijc->ic', edges, dist)
    out  = agg + dw @ w1d.T + x @ w2.T     (w1d = w1[:, 128:130])

Sharding: rows i (targets) split across 8 NeuronCores; x/w1/w2 replicated.

Device strategy (tolerance is 2e-2 rel-L2; this kernel lands ~5e-3):
  - edges stream as bf16, dist channels as fp8-e4m3 (dist only feeds the
    small dw term).  Per-core HBM traffic: ~100 MB fp32 -> ~34 MB.
  - edges/dist are PRE-TRANSPOSED on the host to [j, i] layout and
    PAIR-PACKED: two 128-row j-tiles share one DMA so every per-partition
    run is 4 KB (2 KB runs measured only ~250 GB/s; 4 KB sustains ~360)
    and the HWDGE trigger count halves.
  - out^T[f, i] = sum_j xmsg[j, f] * E^T[j, i] accumulates in a single
    [128, 1024] fp32 PSUM tile via 512-col bf16 matmuls (1 cycle/col;
    one matmul may not span a PSUM bank).
  - dist term: DVE computes prod = E^T (*) D^T for both channels of both
    tiles of a pair in ONE tensor_tensor (broadcast AP on E^T, bf16 2x
    perf mode), and the PE folds the j-reduction AND the w1d GEMM into
    the same PSUM accumulation via broadcast-w1d stationaries
    (w1db_c[j, f] = w1d[f, c] for all j), batched across pairs so one
    LDWEIGHTS covers 8 dist matmuls.
  - fp8->bf16 upconversion of dist: every CAST_EVERY-th pair rides a
    GPSIMD SWDGE dma that casts in-flight (fp8 read from HBM, bf16
    written to SBUF -- zero engine cost); the rest run on the
    otherwise-idle ScalarE activation pipe.  This balances ScalarE,
    DVE and the two DMA constraints (HBM ~358 GB/s, SBUF AXI ~435 GB/s)
    all near the ~95 us HBM floor.
"""

import os

import numpy as np
import ml_dtypes

import concourse.bacc as bacc
import concourse.mybir as mybir
from concourse.tile import TileContext

F32 = mybir.dt.float32
BF16 = mybir.dt.bfloat16
FP8 = mybir.dt.float8e4
P = 128

# problem dims (hardcoded per contract)
N_FULL = 8192
F_IN = 128
F_OUT = 128
N_CORES = 8

# tunables
CAST_EVERY = 3      # every k-th PAIR loads dist via casting SWDGE dma
DIST_BATCH = 2      # pairs sharing one w1db stationary load
STREAM_BUFS = 4     # pair-tiles buffered (4 pairs = 8 j-tiles of runway)

LAST_RESULT = None  # BassKernelResults of the most recent kernel() call


def build(n=N_FULL, rows=N_FULL // N_CORES):
    """Build the per-core SPMD Bass program.

    n:    number of source nodes j (contraction dim, partition-tiled)
    rows: number of target rows i this core handles (free dim of out^T)
    """
    f = F_IN
    assert n % (2 * P) == 0 and rows == 1024
    n_pair = n // (2 * P)  # pair-packed j-tiles streamed from HBM
    XCH = 1024             # x^T prologue piece width (j columns)

    nc = bacc.Bacc()
    eP_d = nc.declare_dram_parameter("eP", [n_pair, P, 2, rows], BF16,
                                     isOutput=False)
    dP_d = nc.declare_dram_parameter("dP", [n_pair, P, 2, 2, rows], FP8,
                                     isOutput=False)
    xT_d = nc.declare_dram_parameter("xT", [f, n], BF16, isOutput=False)
    xTs_d = nc.declare_dram_parameter("xT_self", [f, rows], BF16, isOutput=False)
    w1xT_d = nc.declare_dram_parameter("w1xT", [f, F_OUT], BF16, isOutput=False)
    w2T_d = nc.declare_dram_parameter("w2T", [f, F_OUT], BF16, isOutput=False)
    w1db_d = nc.declare_dram_parameter("w1db", [P, 2, F_OUT], BF16, isOutput=False)
    o_d = nc.declare_dram_parameter("outT", [F_OUT, rows], F32, isOutput=True)

    with TileContext(nc) as tc:
        with (
            tc.tile_pool(name="const", bufs=1) as cpool,
            tc.tile_pool(name="stream", bufs=2) as pool,
            tc.tile_pool(name="psum", bufs=2, space="PSUM") as pp,
        ):
            def load_pair(q):
                """Returns (et2, db_or_d8, is_cast). et2: [P, 2jt, rows]."""
                et2 = pool.tile([P, 2, rows], BF16, tag="E", bufs=STREAM_BUFS,
                                name=f"et{q}")
                nc.sync.dma_start(et2, eP_d[q])
                if q % CAST_EVERY == CAST_EVERY - 1:
                    # casting SWDGE load: fp8 in HBM -> bf16 in SBUF
                    db = pool.tile([P, 2, 2, rows], BF16, tag="DC", bufs=2,
                                   name=f"dbc{q}")
                    nc.gpsimd.dma_start(db, dP_d[q])
                    return et2, db, True
                d8 = pool.tile([P, 2, 2, rows], FP8, tag="D", bufs=STREAM_BUFS,
                               name=f"d8_{q}")
                nc.sync.dma_start(d8, dP_d[q])
                return et2, d8, False

            # issue the first pairs' loads before anything else so the
            # rings start streaming at t=0
            pre = {q: load_pair(q) for q in range(2)}

            # ---------------- prologue (scalar ring) ----------------
            xTs_sb = cpool.tile([f, rows], BF16)
            nc.scalar.dma_start(xTs_sb, xTs_d[:, :])
            w2T = cpool.tile([f, F_OUT], BF16)
            nc.scalar.dma_start(w2T, w2T_d[:, :])
            w1xT = cpool.tile([f, F_OUT], BF16)
            nc.scalar.dma_start(w1xT, w1xT_d[:, :])
            w1db = cpool.tile([P, 2, F_OUT], BF16)
            nc.scalar.dma_start(w1db, w1db_d[:, :, :])

            # x^T pieces so the xmsg GEMM can start as soon as each lands
            xTp = []
            for b in range(n // XCH):
                t = cpool.tile([f, XCH], BF16, name=f"xTp{b}")
                nc.scalar.dma_start(t, xT_d[:, b * XCH : (b + 1) * XCH])
                xTp.append(t)

            # xmsg[j, f] stationary chunks: 4 matmuls per PSUM bank then
            # one batched bf16 copy, alternating copy engine
            xmsg = cpool.tile([P, n // P, f], BF16)
            for qq in range(n // P // 4):
                xm = pp.tile([P, 4, f], F32, tag="xstage", bufs=4)
                for r in range(4):
                    ch = 4 * qq + r
                    b, off = divmod(ch * P, XCH)
                    nc.tensor.matmul(
                        xm[:, r],
                        xTp[b][:, off : off + P],
                        w1xT,
                        start=True,
                        stop=True,
                    )
                if qq % 2 == 0:
                    nc.vector.tensor_copy(xmsg[:, 4 * qq : 4 * qq + 4], xm)
                else:
                    nc.scalar.copy(xmsg[:, 4 * qq : 4 * qq + 4], xm)

            # ---------------- main loop ----------------
            agg = pp.tile([P, rows], F32, tag="agg", bufs=1, name="agg")
            # self-connection term opens each half's accumulation group
            # (a matmul may not span a PSUM bank -> 512-col halves)
            for h in range(2):
                sl = slice(h * 512, (h + 1) * 512)
                nc.tensor.matmul(
                    agg[:, sl], w2T, xTs_sb[:, sl], start=True, stop=False
                )

            def do_pair(q, et2, dd, is_cast):
                """DVE/Scalar work for one pair; returns prod tile."""
                if is_cast:
                    db = dd  # already bf16
                else:
                    db = pool.tile([P, 2, 2, rows], BF16, tag="DB", bufs=3,
                                   name=f"db{q}")
                    nc.scalar.copy(db, dd)
                prod = pool.tile([P, 2, 2, rows], BF16, tag="PR",
                                 bufs=2 * DIST_BATCH, name=f"prod{q}")
                ebc = et2[:, :, None, :].broadcast_to((P, 2, 2, rows))
                nc.vector.tensor_tensor(prod, ebc, db, op=mybir.AluOpType.mult)
                return prod

            for g0 in range(0, n_pair, DIST_BATCH):
                group = range(g0, min(g0 + DIST_BATCH, n_pair))
                prods = []
                for q in group:
                    et2, dd, is_cast = pre.pop(q) if q in pre else load_pair(q)
                    prods.append(do_pair(q, et2, dd, is_cast))
                    # agg matmuls (stationary = xmsg chunk, reused for
                    # both 512-col halves)
                    for t in range(2):
                        for h in range(2):
                            sl = slice(h * 512, (h + 1) * 512)
                            nc.tensor.matmul(
                                agg[:, sl],
                                xmsg[:, 2 * q + t],
                                et2[:, t, sl],
                                start=False,
                                stop=False,
                            )
                last_group = g0 + DIST_BATCH >= n_pair
                for c in range(2):
                    for k, q in enumerate(group):
                        for t in range(2):
                            for h in range(2):
                                sl = slice(h * 512, (h + 1) * 512)
                                nc.tensor.matmul(
                                    agg[:, sl],
                                    w1db[:, c],
                                    prods[k][:, t, c, sl],
                                    start=False,
                                    stop=last_group and c == 1
                                    and q == n_pair - 1 and t == 1,
                                )

            # ---------------- epilogue ----------------
            out_sb = pool.tile([P, rows], F32, tag="osb", bufs=1)
            for h in range(2):
                sl = slice(h * 512, (h + 1) * 512)
                nc.scalar.copy(out_sb[:, sl], agg[:, sl])
                nc.sync.dma_start(o_d[:, sl], out_sb[:, sl])

    nc.compile()
    return nc


def _prep_inputs(inputs, n, rows_per_core, n_cores):
    """Host-side shard + layout + dtype prep (numpy only)."""
    bf16 = ml_dtypes.bfloat16
    # e4m3fn: bit-identical to TRN FP8_EXP4 for |x| <= 240 (all our data),
    # and the fn variant is the one the PJRT plugin accepts as input type
    fp8 = ml_dtypes.float8_e4m3fn
    x = np.asarray(inputs["x"], dtype=np.float32)
    edges = np.asarray(inputs["edges"], dtype=np.float32)
    dist = np.asarray(inputs["distance_matrix"], dtype=np.float32)
    w1 = np.asarray(inputs["w1"], dtype=np.float32)
    w2 = np.asarray(inputs["w2"], dtype=np.float32)
    f = x.shape[1]
    R = rows_per_core

    xT = np.ascontiguousarray(x.T).astype(bf16)            # [f, n]
    w1xT = np.ascontiguousarray(w1[:, :f].T).astype(bf16)  # [k, F_OUT]
    w2T = np.ascontiguousarray(w2.T).astype(bf16)          # [k, F_OUT]
    # broadcast-w1d stationaries: w1db[j, c, fout] = w1[fout, f+c]
    w1db = np.ascontiguousarray(
        np.broadcast_to(w1[:, f : f + 2].T[None, :, :], (P, 2, w1.shape[0]))
    ).astype(bf16)

    in_maps = []
    for c in range(n_cores):
        i0 = c * R
        i1 = i0 + R
        # E^T slice [n, R], pair-packed to [n/256, 128, 2, R] so each
        # partition row carries 4 KB contiguous
        eT = edges[i0:i1].T.astype(bf16)
        eP = np.ascontiguousarray(
            eT.reshape(n // (2 * P), 2, P, R).transpose(0, 2, 1, 3)
        )
        # D^T channel-packed [n, 2, R] (dT[j, c, i] = dist[i, j, c]),
        # pair-packed to [n/256, 128, 2, 2, R]
        dT = dist[i0:i1].transpose(1, 2, 0).astype(fp8)
        dP = np.ascontiguousarray(
            dT.reshape(n // (2 * P), 2, P, 2, R).transpose(0, 2, 1, 3, 4)
        )
        in_maps.append(
            {
                "eP": eP,
                "dP": dP,
                "xT": xT,
                "xT_self": np.ascontiguousarray(xT[:, i0:i1]),
                "w1xT": w1xT,
                "w2T": w2T,
                "w1db": w1db,
            }
        )
    return in_maps


def _run(inputs, n, rows_per_core, n_cores, trace=False):
    from concourse.bass_utils import run_bass_kernel_spmd

    in_maps = _prep_inputs(inputs, n, rows_per_core, n_cores)
    nc = build(n=n, rows=rows_per_core)
    res = run_bass_kernel_spmd(nc, in_maps, core_ids=list(range(n_cores)), trace=trace)

    global LAST_RESULT
    LAST_RESULT = res

    out = np.concatenate([r["outT"].T for r in res.results], axis=0)
    return np.ascontiguousarray(out, dtype=np.float32)


def kernel(**inputs) -> np.ndarray:
    trace = os.environ.get("KERNEL_TRACE", "0") == "1"
    return _run(
        inputs,
        n=N_FULL,
        rows_per_core=N_FULL // N_CORES,
        n_cores=N_CORES,
        trace=trace,
    )


# revision 8
# speedup vs baseline: 2.1215x; 1.0647x over previous
"""Trainium2 Bass kernel for a GCN message-passing layer.

Reference computation (per node i):
    out[i] = sum_j edges[i,j] * (w1 @ concat(x[j], dist[i,j])) + w2 @ x[i]
which factors into:
    xmsg = x @ w1x.T                       (w1x = w1[:, :128])
    agg  = edges @ xmsg                    (big GEMM, contraction over j)
    dw   = einsum('ij,ijc->ic', edges, dist)
    out  = agg + dw @ w1d.T + x @ w2.T     (w1d = w1[:, 128:130])

Sharding: rows i (targets) split across 8 NeuronCores; x/w1/w2 replicated.

Device strategy (tolerance is 2e-2 rel-L2; this kernel lands ~5e-3):
  - edges stream as bf16, dist channels as fp8-e4m3 (dist only feeds the
    small dw term).  Per-core HBM traffic: ~100 MB fp32 -> ~34 MB.
  - edges/dist are PRE-TRANSPOSED on the host to [j, i] layout and
    PAIR-PACKED: two 128-row j-tiles share one DMA so every per-partition
    run is 4 KB (2 KB runs measured only ~250 GB/s) and the HWDGE
    trigger count halves.
  - out^T[f, i] = sum_j xmsg[j, f] * E^T[j, i] accumulates in a single
    [128, 1024] fp32 PSUM tile via 512-col bf16 matmuls (1 cycle/col;
    one matmul may not span a PSUM bank).
  - dist term: DVE computes prod = E^T (*) D^T for both channels of both
    tiles of a pair in ONE tensor_tensor (broadcast AP on E^T, bf16 2x
    perf mode), and the PE folds the j-reduction AND the w1d GEMM into
    the same PSUM accumulation via broadcast-w1d stationaries
    (w1db_c[j, f] = w1d[f, c] for all j), batched across pairs so one
    LDWEIGHTS covers 8 dist matmuls.
  - fp8->bf16 upconversion of dist: ~1/3 of pairs (incl. the last two,
    for a short tail) ride a GPSIMD SWDGE dma that casts in-flight (fp8
    read from HBM, bf16 written to SBUF -- zero engine cost); the rest
    run on the otherwise-idle ScalarE activation pipe.
  - the xmsg prologue is INTERLEAVED into the main loop (one 4-chunk
    PSUM batch per pair-group, 2 groups of lookahead) so neither the
    ScalarE nor the DVE queue is head-of-line blocked at t=0 and the
    stream starts consuming immediately.
"""

import os

import numpy as np
import ml_dtypes

import concourse.bacc as bacc
import concourse.mybir as mybir
from concourse.tile import TileContext

F32 = mybir.dt.float32
BF16 = mybir.dt.bfloat16
FP8 = mybir.dt.float8e4
P = 128

# problem dims (hardcoded per contract)
N_FULL = 8192
F_IN = 128
F_OUT = 128
N_CORES = 8

# tunables
CAST_EVERY = 3      # every k-th pair loads dist via casting SWDGE dma
DIST_BATCH = 2      # pairs sharing one w1db stationary load
STREAM_BUFS = 5     # pair-tiles buffered per stream tag

LAST_RESULT = None  # BassKernelResults of the most recent kernel() call


def build(n=N_FULL, rows=N_FULL // N_CORES):
    """Build the per-core SPMD Bass program.

    n:    number of source nodes j (contraction dim, partition-tiled)
    rows: number of target rows i this core handles (free dim of out^T)
    """
    f = F_IN
    assert n % (2 * P) == 0 and rows == 1024
    n_pair = n // (2 * P)  # pair-packed j-tiles streamed from HBM
    n_grp = n_pair // DIST_BATCH
    XCH = n // 2           # x^T prologue piece width (j columns)

    nc = bacc.Bacc()
    eP_d = nc.declare_dram_parameter("eP", [n_pair, P, 2, rows], BF16,
                                     isOutput=False)
    dP_d = nc.declare_dram_parameter("dP", [n_pair, P, 2, 2, rows], FP8,
                                     isOutput=False)
    xT_d = nc.declare_dram_parameter("xT", [f, n], BF16, isOutput=False)
    xTs_d = nc.declare_dram_parameter("xT_self", [f, rows], BF16, isOutput=False)
    w1xT_d = nc.declare_dram_parameter("w1xT", [f, F_OUT], BF16, isOutput=False)
    w2T_d = nc.declare_dram_parameter("w2T", [f, F_OUT], BF16, isOutput=False)
    w1db_d = nc.declare_dram_parameter("w1db", [P, 2, F_OUT], BF16, isOutput=False)
    o_d = nc.declare_dram_parameter("outT", [F_OUT, rows], F32, isOutput=True)

    def is_cast(q):
        return q % CAST_EVERY == CAST_EVERY - 1 or q >= n_pair - 2

    with TileContext(nc) as tc:
        with (
            tc.tile_pool(name="const", bufs=1) as cpool,
            tc.tile_pool(name="stream", bufs=2) as pool,
            tc.tile_pool(name="psum", bufs=2, space="PSUM") as pp,
        ):
            def load_pair(q):
                """Returns (et2, db_or_d8). et2: [P, 2jt, rows]."""
                et2 = pool.tile([P, 2, rows], BF16, tag="E", bufs=STREAM_BUFS,
                                name=f"et{q}")
                nc.sync.dma_start(et2, eP_d[q])
                if is_cast(q):
                    # casting SWDGE load: fp8 in HBM -> bf16 in SBUF
                    db = pool.tile([P, 2, 2, rows], BF16, tag="DC", bufs=2,
                                   name=f"dbc{q}")
                    nc.gpsimd.dma_start(db, dP_d[q])
                    return et2, db
                d8 = pool.tile([P, 2, 2, rows], FP8, tag="D", bufs=STREAM_BUFS,
                               name=f"d8_{q}")
                nc.sync.dma_start(d8, dP_d[q])
                return et2, d8

            # issue the first pairs' loads before anything else so the
            # rings start streaming at t=0
            pre = {q: load_pair(q) for q in range(4)}

            # ---------------- prologue ----------------
            # x^T halves ride the scalar HWDGE ring (fast; only 2 trigger
            # instructions ahead of the upconverts in the scalar queue);
            # small weights ride the SWDGE ring, which is otherwise idle
            xTp = []
            for b in range(n // XCH):
                t = cpool.tile([f, XCH], BF16, name=f"xTp{b}")
                nc.scalar.dma_start(t, xT_d[:, b * XCH : (b + 1) * XCH])
                xTp.append(t)
            w2T = cpool.tile([f, F_OUT], BF16)
            nc.gpsimd.dma_start(w2T, w2T_d[:, :])
            w1xT = cpool.tile([f, F_OUT], BF16)
            nc.gpsimd.dma_start(w1xT, w1xT_d[:, :])
            w1db = cpool.tile([P, 2, F_OUT], BF16)
            nc.gpsimd.dma_start(w1db, w1db_d[:, :, :])
            xTs_sb = cpool.tile([f, rows], BF16)
            nc.gpsimd.dma_start(xTs_sb, xTs_d[:, :])

            xmsg = cpool.tile([P, n // P, f], BF16)

            def xmsg_stage(qq):
                """One 4-chunk xmsg batch: 4 matmuls + one copy."""
                xm = pp.tile([P, 4, f], F32, tag="xstage", bufs=4)
                for r in range(4):
                    ch = 4 * qq + r
                    b, off = divmod(ch * P, XCH)
                    nc.tensor.matmul(
                        xm[:, r],
                        xTp[b][:, off : off + P],
                        w1xT,
                        start=True,
                        stop=True,
                    )
                if qq % 2 == 0:
                    nc.vector.tensor_copy(xmsg[:, 4 * qq : 4 * qq + 4], xm)
                else:
                    nc.scalar.copy(xmsg[:, 4 * qq : 4 * qq + 4], xm)

            # ---------------- main loop ----------------
            agg = pp.tile([P, rows], F32, tag="agg", bufs=1, name="agg")
            # self-connection term opens each half's accumulation group
            # (a matmul may not span a PSUM bank -> 512-col halves)
            for h in range(2):
                sl = slice(h * 512, (h + 1) * 512)
                nc.tensor.matmul(
                    agg[:, sl], w2T, xTs_sb[:, sl], start=True, stop=False
                )

            def do_pair(q, et2, dd):
                """DVE/Scalar work for one pair; returns prod tile."""
                if is_cast(q):
                    db = dd  # already bf16 via casting dma
                else:
                    db = pool.tile([P, 2, 2, rows], BF16, tag="DB", bufs=3,
                                   name=f"db{q}")
                    nc.scalar.copy(db, dd)
                prod = pool.tile([P, 2, 2, rows], BF16, tag="PR",
                                 bufs=2 * DIST_BATCH, name=f"prod{q}")
                ebc = et2[:, :, None, :].broadcast_to((P, 2, 2, rows))
                nc.vector.tensor_tensor(prod, ebc, db, op=mybir.AluOpType.mult)
                return prod

            # xmsg batch g feeds pair-group g; keep 2 groups of lookahead
            xmsg_stage(0)
            xmsg_stage(1)

            for g in range(n_grp):
                group = range(g * DIST_BATCH, (g + 1) * DIST_BATCH)
                prods = []
                for q in group:
                    et2, dd = pre.pop(q) if q in pre else load_pair(q)
                    prods.append(do_pair(q, et2, dd))
                    # agg matmuls (stationary = xmsg chunk, reused for
                    # both 512-col halves)
                    for t in range(2):
                        for h in range(2):
                            sl = slice(h * 512, (h + 1) * 512)
                            nc.tensor.matmul(
                                agg[:, sl],
                                xmsg[:, 2 * q + t],
                                et2[:, t, sl],
                                start=False,
                                stop=False,
                            )
                if g + 2 < n_grp:
                    xmsg_stage(g + 2)
                for c in range(2):
                    for k, q in enumerate(group):
                        for t in range(2):
                            for h in range(2):
                                sl = slice(h * 512, (h + 1) * 512)
                                nc.tensor.matmul(
                                    agg[:, sl],
                                    w1db[:, c],
                                    prods[k][:, t, c, sl],
                                    start=False,
                                    stop=c == 1 and q == n_pair - 1 and t == 1,
                                )

            # ---------------- epilogue ----------------
            out_sb = pool.tile([P, rows], F32, tag="osb", bufs=1)
            for h in range(2):
                sl = slice(h * 512, (h + 1) * 512)
                nc.scalar.copy(out_sb[:, sl], agg[:, sl])
                nc.sync.dma_start(o_d[:, sl], out_sb[:, sl])

    nc.compile()
    return nc


def _prep_inputs(inputs, n, rows_per_core, n_cores):
    """Host-side shard + layout + dtype prep (numpy only)."""
    bf16 = ml_dtypes.bfloat16
    # e4m3fn: bit-identical to TRN FP8_EXP4 for |x| <= 240 (all our data),
    # and the fn variant is the one the PJRT plugin accepts as input type
    fp8 = ml_dtypes.float8_e4m3fn
    x = np.asarray(inputs["x"], dtype=np.float32)
    edges = np.asarray(inputs["edges"], dtype=np.float32)
    dist = np.asarray(inputs["distance_matrix"], dtype=np.float32)
    w1 = np.asarray(inputs["w1"], dtype=np.float32)
    w2 = np.asarray(inputs["w2"], dtype=np.float32)
    f = x.shape[1]
    R = rows_per_core

    xT = np.ascontiguousarray(x.T).astype(bf16)            # [f, n]
    w1xT = np.ascontiguousarray(w1[:, :f].T).astype(bf16)  # [k, F_OUT]
    w2T = np.ascontiguousarray(w2.T).astype(bf16)          # [k, F_OUT]
    # broadcast-w1d stationaries: w1db[j, c, fout] = w1[fout, f+c]
    w1db = np.ascontiguousarray(
        np.broadcast_to(w1[:, f : f + 2].T[None, :, :], (P, 2, w1.shape[0]))
    ).astype(bf16)

    in_maps = []
    for c in range(n_cores):
        i0 = c * R
        i1 = i0 + R
        # E^T slice [n, R], pair-packed to [n/256, 128, 2, R] so each
        # partition row carries 4 KB contiguous
        eT = edges[i0:i1].T.astype(bf16)
        eP = np.ascontiguousarray(
            eT.reshape(n // (2 * P), 2, P, R).transpose(0, 2, 1, 3)
        )
        # D^T channel-packed [n, 2, R] (dT[j, c, i] = dist[i, j, c]),
        # pair-packed to [n/256, 128, 2, 2, R]
        dT = dist[i0:i1].transpose(1, 2, 0).astype(fp8)
        dP = np.ascontiguousarray(
            dT.reshape(n // (2 * P), 2, P, 2, R).transpose(0, 2, 1, 3, 4)
        )
        in_maps.append(
            {
                "eP": eP,
                "dP": dP,
                "xT": xT,
                "xT_self": np.ascontiguousarray(xT[:, i0:i1]),
                "w1xT": w1xT,
                "w2T": w2T,
                "w1db": w1db,
            }
        )
    return in_maps


def _run(inputs, n, rows_per_core, n_cores, trace=False):
    from concourse.bass_utils import run_bass_kernel_spmd

    in_maps = _prep_inputs(inputs, n, rows_per_core, n_cores)
    nc = build(n=n, rows=rows_per_core)
    res = run_bass_kernel_spmd(nc, in_maps, core_ids=list(range(n_cores)), trace=trace)

    global LAST_RESULT
    LAST_RESULT = res

    out = np.concatenate([r["outT"].T for r in res.results], axis=0)
    return np.ascontiguousarray(out, dtype=np.float32)


def kernel(**inputs) -> np.ndarray:
    trace = os.environ.get("KERNEL_TRACE", "0") == "1"
    return _run(
        inputs,
        n=N_FULL,
        rows_per_core=N_FULL // N_CORES,
        n_cores=N_CORES,
        trace=trace,
    )
